# revision 1
# baseline (speedup 1.0000x reference)
"""YOLOv2-style loss (nn_CostYoloV2) on 8 Trainium2 NeuronCores.

Sharding:
  * per-(batch,truth) losses (obj / class / coords): data-parallel over batch,
    8 batches per core.  Only the 115 channels actually consumed downstream
    (per-anchor w,h,obj,classes = ch n*25+2..24) are DMA'd, as one affine
    3D access pattern per batch.  Channel gathers at the truth cells happen
    on-chip (gpsimd indirect_copy), then a PE transpose yields [bt, ch]
    tiles for the vector/scalar-engine loss math.
  * the noobj term and the coord warmup term only involve batch 63 - those
    are sharded over the 1024 spatial cells (128 cells per core).
  * scalar partials are returned per core and combined on the host (the
    "all-reduce" of the loss terms).

Perf structure: per-batch DMAs alternate between the two HWDGE queues
(sync + scalar engines) so transfers pipeline; gather/transpose/math are
emitted per batch-pair so the tile scheduler overlaps them with the DMA
stream; elementwise work is split between the Vector and Scalar engines.
"""
import numpy as np
from contextlib import ExitStack

import concourse.bass as bass
import concourse.bacc as bacc
import concourse.mybir as mybir
import concourse.tile as tile
from concourse.bass_utils import run_bass_kernel_spmd
from concourse.masks import make_identity

F32 = mybir.dt.float32
U16 = mybir.dt.uint16
Alu = mybir.AluOpType
Ax = mybir.AxisListType
Act = mybir.ActivationFunctionType

B, NUM, CLASSES, AL = 64, 5, 20, 25
H = W = 32
HW = H * W
T = 50
NCORES = 8
BL = B // NCORES      # local batches per core
TILES = BL // 2       # bt tiles of 2 batches x 50 truths
ROWS = 2 * T          # 100 partitions per bt tile
CELLS = HW // NCORES  # 128 noobj cells per core
CH = 23               # channels kept per anchor (25 minus unused pred x,y)
NCH = NUM * CH        # 115
ICOLS = -(-T // 16)  # 4 u16 index columns per batch (wrapped by 16)

_CACHED = {}


def _build_program():
    nc = bacc.Bacc()
    xin = nc.declare_dram_parameter("xin", [BL, NUM, CH, HW], F32, isOutput=False)
    gidx = nc.declare_dram_parameter("gidx", [128, BL * ICOLS], U16, isOutput=False)
    tv = nc.declare_dram_parameter("tv", [ROWS, TILES * 28], F32, isOutput=False)
    p63 = nc.declare_dram_parameter("p63", [128, 21], F32, isOutput=False)
    tb63 = nc.declare_dram_parameter("tb63", [128, 250], F32, isOutput=False)
    ob4 = nc.declare_dram_parameter("ob4", [ROWS, TILES * 3], F32, isOutput=True)
    onw = nc.declare_dram_parameter("onw", [128, 3], F32, isOutput=True)

    with tile.TileContext(nc) as tc, ExitStack() as ctx:
        const = ctx.enter_context(tc.tile_pool(name="const", bufs=1))
        xpool = ctx.enter_context(tc.tile_pool(name="xp", bufs=4))
        work = ctx.enter_context(tc.tile_pool(name="wk", bufs=4))
        psum = ctx.enter_context(tc.tile_pool(name="ps", bufs=4, space="PSUM"))

        ident = const.tile([128, 128], F32)
        make_identity(nc, ident[:])

        # ---------------- input DMAs (two HWDGE queues) -----------------------
        gx = const.tile([128, BL * ICOLS], U16)
        nc.sync.dma_start(gx[:], gidx[:])
        tb = const.tile([128, 250], F32)
        nc.sync.dma_start(tb[:], tb63[:])
        pp = const.tile([128, 21], F32)
        nc.scalar.dma_start(pp[:], p63[:])
        tvt = const.tile([ROWS, TILES * 28], F32)
        nc.scalar.dma_start(tvt[:], tv[:])

        xts = []
        for k in range(TILES):
            xt = xpool.tile([128, 2 * HW], F32, tag="xt", name=f"xt{k}")
            xts.append(xt)
        for b in range(BL):
            eng = nc.sync if b % 2 == 0 else nc.scalar
            xt = xts[b // 2]
            half = (b % 2) * HW
            eng.dma_start(xt[0:NCH, half:half + HW], xin[b])

        # ---------------- noobj (batch 63, this core's 128 cells) -------------
        pv = pp[:, 0:20].rearrange("p (n c) -> p n c", c=4)
        xc, yc, wc, hc = pv[:, :, 0], pv[:, :, 1], pv[:, :, 2], pv[:, :, 3]
        corn = const.tile([128, 25], F32)  # al|au|ar|ad|hap blocks of 5
        al, au = corn[:, 0:5], corn[:, 5:10]
        ar, ad = corn[:, 10:15], corn[:, 15:20]
        hap = corn[:, 20:25]
        nc.vector.scalar_tensor_tensor(al, wc, -0.5, xc, Alu.mult, Alu.add)
        nc.vector.scalar_tensor_tensor(au, hc, -0.5, yc, Alu.mult, Alu.add)
        nc.vector.scalar_tensor_tensor(ar, wc, 0.5, xc, Alu.mult, Alu.add)
        nc.vector.scalar_tensor_tensor(ad, hc, 0.5, yc, Alu.mult, Alu.add)
        nc.vector.scalar_tensor_tensor(hap, wc, 0.5, hc, Alu.mult, Alu.mult)

        def bc_p(apv):  # [128, 5] -> [128, 5, 50]
            return apv.broadcast_to([128, 5, T])

        def bc_t(col):  # tb block [128, 50] -> [128, 5, 50]
            return tb[:, col * T:(col + 1) * T].rearrange(
                "p (o f) -> p o f", o=1).broadcast_to([128, 5, T])

        def w3(tl):  # [128, 250] tile -> [128, 5, 50] view
            return tl[:].rearrange("p (n t) -> p n t", t=T)

        m1 = const.tile([128, 5 * T], F32)
        m2 = const.tile([128, 5 * T], F32)
        iw = const.tile([128, 5 * T], F32)
        ih = const.tile([128, 5 * T], F32)
        nc.vector.tensor_tensor(w3(m1), bc_t(2), bc_p(ar), Alu.min)
        nc.vector.tensor_tensor(w3(m2), bc_t(0), bc_p(al), Alu.max)
        nc.vector.tensor_tensor(iw[:], m1[:], m2[:], Alu.subtract)
        nc.scalar.activation(iw[:], iw[:], Act.Relu)
        nc.vector.tensor_tensor(w3(m1), bc_t(3), bc_p(ad), Alu.min)
        nc.vector.tensor_tensor(w3(m2), bc_t(1), bc_p(au), Alu.max)
        nc.vector.tensor_tensor(ih[:], m1[:], m2[:], Alu.subtract)
        nc.scalar.activation(ih[:], ih[:], Act.Relu)
        nc.vector.tensor_tensor(iw[:], iw[:], ih[:], Alu.mult)   # inter
        nc.scalar.mul(iw[:], iw[:], 1.5)
        nc.vector.tensor_tensor(w3(iw), w3(iw), bc_p(hap), Alu.subtract)
        nc.vector.tensor_tensor(w3(m1), w3(iw), bc_t(4), Alu.is_gt)  # mask
        anyt = const.tile([128, 6], F32)
        nc.vector.tensor_reduce(anyt[:, 0:5], w3(m1), Ax.X, Alu.max)
        smk = const.tile([128, 1], F32)
        nc.vector.tensor_reduce(smk[:], anyt[:, 0:5], Ax.X, Alu.add)
        oq2 = const.tile([128, 1], F32)
        nc.scalar.square(oq2[:], pp[:, 20:21])
        now = const.tile([128, 3], F32)
        nc.vector.tensor_scalar(smk[:], smk[:], -1.0, float(NUM), Alu.mult, Alu.add)
        nc.vector.tensor_tensor(now[:, 0:1], smk[:], oq2[:], Alu.mult)

        # ---------------- warm coords (batch 63 cells) ------------------------
        scr20 = const.tile([128, 20], F32)
        nc.scalar.activation(scr20[:], pp[:, 0:20], Act.Square,
                             accum_out=now[:, 1:2])
        nc.vector.tensor_reduce(now[:, 2:3], pv[:, :, 0:2], Ax.XY, Alu.add)
        nc.sync.dma_start(onw[:], now[:])

        # ---------------- per-(b,t) losses, one pair (2 batches) at a time ----
        ob = const.tile([ROWS, TILES * 3], F32)
        for k in range(TILES):
            cl = work.tile([128, ROWS], F32, tag="cl", name=f"cl{k}")
            for h in range(2):
                b = 2 * k + h
                nc.gpsimd.indirect_copy(cl[:, h * T:(h + 1) * T],
                                        xts[k][:, h * HW:(h + 1) * HW],
                                        gx[:, b * ICOLS:(b + 1) * ICOLS], True)
            tp = psum.tile([ROWS, 128], F32, space="PSUM", tag="tp",
                           name=f"tp{k}")
            nc.tensor.transpose(tp[:], cl[:], ident[:])
            cell = work.tile([ROWS, NCH], F32, tag="cell", name=f"cell{k}")
            nc.scalar.copy(cell[:], tp[:, 0:NCH])

            tvk = tvt[:, k * 28:(k + 1) * 28]
            wt, ht = tvk[:, 0:1], tvk[:, 1:2]
            at, tw, th = tvk[:, 2:3], tvk[:, 3:4], tvk[:, 4:5]
            s2v, txy2, vld = tvk[:, 5:6], tvk[:, 6:7], tvk[:, 7:8]
            cv = cell[:].rearrange("p (n c) -> p n c", c=CH)
            wv, hv = cv[:, :, 0], cv[:, :, 1]

            t1 = work.tile([ROWS, 5], F32, tag="t1", name=f"t1_{k}")
            t2 = work.tile([ROWS, 5], F32, tag="t2", name=f"t2_{k}")
            t3 = work.tile([ROWS, 5], F32, tag="t3", name=f"t3_{k}")
            t4 = work.tile([ROWS, 5], F32, tag="t4", name=f"t4_{k}")
            sc = work.tile([ROWS, 8], F32, tag="sc", name=f"sc{k}")
            # sc cols: 0 m | 1 wb | 2 hb | 3 sq1 | 4 q1 | 5 q2 | 6 od
            nc.vector.tensor_scalar(t1[:], wv, wt, 0.0, Alu.min, Alu.max)
            nc.vector.tensor_scalar(t2[:], hv, ht, 0.0, Alu.min, Alu.max)
            nc.vector.tensor_tensor(t3[:], t1[:], t2[:], Alu.mult)    # inter
            nc.vector.tensor_tensor(t4[:], wv, hv, Alu.mult)          # wp*hp
            nc.vector.scalar_tensor_tensor(t4[:], t3[:], -1.0, t4[:],
                                           Alu.mult, Alu.add)
            nc.vector.tensor_scalar(t4[:], t4[:], at, 1e-12, Alu.add, Alu.max)
            nc.vector.reciprocal(t4[:], t4[:])
            nc.vector.tensor_tensor(t3[:], t3[:], t4[:], Alu.mult)    # iou
            nc.vector.tensor_reduce(sc[:, 0:1], t3[:], Ax.X, Alu.max)
            nc.vector.tensor_scalar(t1[:], t3[:], sc[:, 0:1], None, Alu.is_ge)
            nc.vector.scalar_tensor_tensor(t2[:], t1[:], 1.0, wv, Alu.mult,
                                           Alu.mult, accum_out=sc[:, 1:2])
            nc.vector.scalar_tensor_tensor(t2[:], t1[:], 1.0, hv, Alu.mult,
                                           Alu.mult, accum_out=sc[:, 2:3])
            # coords (scalar engine): q1=(tw-wb)^2, q2=(th-hb)^2,
            # ob0 = (q1+q2+txy2)*s2v
            nc.scalar.activation(sc[:, 4:5], sc[:, 1:2], Act.Square,
                                 bias=tw, scale=-1.0)
            nc.scalar.activation(sc[:, 5:6], sc[:, 2:3], Act.Square,
                                 bias=th, scale=-1.0)
            nc.scalar.add(sc[:, 4:5], sc[:, 4:5], sc[:, 5:6])
            nc.scalar.add(sc[:, 4:5], sc[:, 4:5], txy2)
            nc.scalar.mul(ob[:, 3 * k:3 * k + 1], sc[:, 4:5], s2v)
            # obj (scalar engine): ob1 = (obj-1)^2 * vld
            nc.scalar.activation(sc[:, 6:7], cell[:, 2:3], Act.Square,
                                 bias=1.0, scale=-1.0)
            nc.scalar.mul(ob[:, 3 * k + 1:3 * k + 2], sc[:, 6:7], vld)
            # classes
            ca = work.tile([ROWS, CLASSES], F32, tag="ca", name=f"ca{k}")
            cb = work.tile([ROWS, CLASSES], F32, tag="cb", name=f"cb{k}")
            nc.vector.tensor_scalar(ca[:], cell[:, 3:23], t1[:, 0:1], None,
                                    Alu.mult)
            for n in range(1, NUM):
                src, dst = (ca, cb) if n % 2 == 1 else (cb, ca)
                nc.vector.scalar_tensor_tensor(
                    dst[:], cell[:, CH * n + 3:CH * n + 23], t1[:, n:n + 1],
                    src[:], Alu.mult, Alu.add)
            fin = ca if (NUM - 1) % 2 == 0 else cb
            oth = cb if fin is ca else ca
            nc.vector.tensor_tensor(oth[:], tvk[:, 8:28], fin[:], Alu.subtract)
            nc.vector.scalar_tensor_tensor(fin[:], oth[:], 1.0,
                                           oth[:], Alu.mult, Alu.mult,
                                           accum_out=sc[:, 3:4])
            nc.scalar.mul(ob[:, 3 * k + 2:3 * k + 3], sc[:, 3:4], vld)
        nc.sync.dma_start(ob4[:], ob[:])
    nc.finalize()
    return nc


def _wrap_idx(idx):
    """[n] int -> [128, ceil(n/16)] wrapped uint16 (replicated per 16-part group)."""
    n = len(idx)
    cols = -(-n // 16)
    pad = np.zeros(cols * 16, np.uint16)
    pad[:n] = idx
    blk = pad.reshape(cols, 16).T          # [16, cols]
    return np.tile(blk, (8, 1))            # [128, cols]


def _prep(x, truth, anchors):
    f32 = np.float32
    x = np.ascontiguousarray(x, f32)
    truth = np.ascontiguousarray(truth, f32)
    anchors = np.asarray(anchors, f32)

    wt, ht = truth[..., 2], truth[..., 3]
    valid = np.cumprod((wt >= 1e-5).astype(f32), axis=1, dtype=f32)
    i = np.clip((truth[..., 0] * f32(W)).astype(np.int32), 0, W - 1)
    j = np.clip((truth[..., 1] * f32(H)).astype(np.int32), 0, H - 1)
    lin = (j * W + i).astype(np.int64)
    tx = i.astype(f32) / f32(W)
    ty = j.astype(f32) / f32(H)
    tw = np.exp(wt) * anchors[2 * (NUM - 1)] / f32(W)
    th = np.exp(ht) * anchors[2 * (NUM - 1) + 1] / f32(H)
    at = wt * ht
    scale = (f32(2.0) - at).astype(f32)
    s2v = scale * scale * valid
    txy2 = tx * tx + ty * ty
    ct = np.clip(truth[..., 4].astype(np.int32), 0, CLASSES - 1)
    oh = np.eye(CLASSES, dtype=f32)[ct]                      # [B, T, 20]
    tvfull = np.stack([wt, ht, at, tw, th, s2v, txy2, valid], -1)  # [B,T,8]
    tvfull = np.concatenate([tvfull, oh], -1).astype(f32)    # [B,T,28]

    xp63 = x[B - 1].reshape(NUM * AL, HW)
    t63 = truth[B - 1]
    bl = t63[:, 0] - f32(0.5) * t63[:, 2]
    bu = t63[:, 1] - f32(0.5) * t63[:, 3]
    br = t63[:, 0] + f32(0.5) * t63[:, 2]
    bd = t63[:, 1] + f32(0.5) * t63[:, 3]
    hat = f32(0.5) * (t63[:, 2] * t63[:, 3])
    tbrow = np.concatenate([bl, bu, br, bd, hat]).astype(f32)  # [250]
    tb63 = np.tile(tbrow[None, :], (128, 1))

    x5 = x.reshape(B, NUM, AL, HW)
    in_maps = []
    for c in range(NCORES):
        bs = slice(BL * c, BL * (c + 1))
        cells = slice(CELLS * c, CELLS * (c + 1))
        gidx = np.hstack([_wrap_idx(lin[BL * c + b]) for b in range(BL)])
        p63 = np.empty((128, 21), f32)
        for n in range(NUM):
            for cc in range(4):
                p63[:, n * 4 + cc] = xp63[AL * n + cc, cells]
        p63[:, 20] = xp63[4, cells]
        tvc = tvfull[bs].reshape(TILES, 2, T, 28)
        in_maps.append({
            "xin": x5[bs, :, 2:25, :],
            "gidx": gidx.astype(np.uint16),
            "tv": np.ascontiguousarray(
                tvc.transpose(1, 2, 0, 3).reshape(ROWS, TILES * 28)),
            "p63": p63,
            "tb63": tb63,
        })
    return in_maps


def _combine(results):
    obj = sum(float(r["ob4"][:, 1::3].sum(dtype=np.float64)) for r in results)
    cls = sum(float(r["ob4"][:, 2::3].sum(dtype=np.float64)) for r in results)
    coord63 = float(results[NCORES - 1]["ob4"][T:ROWS, 3 * (TILES - 1)]
                    .sum(dtype=np.float64))
    noobj = sum(float(r["onw"][:, 0].sum(dtype=np.float64)) for r in results)
    sq = sum(float(r["onw"][:, 1].sum(dtype=np.float64)) for r in results)
    xy = sum(float(r["onw"][:, 2].sum(dtype=np.float64)) for r in results)
    warm = 0.01 * (sq - xy + 0.5 * NUM * HW)   # +2560: sum of (0.5^2)*2 per (n,cell)
    return np.float32(obj + noobj + warm + coord63 + cls)


def kernel(x, truth, anchors, **_):
    if "nc" not in _CACHED:
        _CACHED["nc"] = _build_program()
    nc = _CACHED["nc"]
    in_maps = _prep(x, truth, anchors)
    res = run_bass_kernel_spmd(nc, in_maps, list(range(NCORES)))
    return _combine(res.results)



# revision 2
# speedup vs baseline: 1.0051x; 1.0051x over previous
"""YOLOv2-style loss (nn_CostYoloV2) on 8 Trainium2 NeuronCores — v2.

Data-parallel over batch (8 batches/core); the noobj term (batch 63 only)
is sharded over the 1024 spatial cells (128 cells/core).

Layout strategy: the truth-cell gather indices are pure host data
(computed from `truth` alone), so the host packs the valid (batch, truth)
rows per core into a [128 part, BLK block, 128 ch] tile and the device
consumes them directly — measured on-device alternatives (streaming all
23 used channels of x + gpsimd indirect_copy + PE transpose: ~26us of DMA
at the 2-queue HWDGE ceiling; swdge dma_gather: ~15us of descriptor
generation; trn2's Pool engine has no elementwise ALU) are all slower.
Rows with valid=0 and all-zero truth slots are pruned host-side (their
loss contributions are provably zero), shrinking both DMA and DVE work.

Per-(b,t) row layout (128 f32): [w x5 anchors | h x5 | obj0 | pad x5 |
cls x20 anchor-innermost | pad x12].  Per-(b,t) scalars ride in a side
tile.  Engine split: DVE runs both the per-(b,t) iou/argmax/coord/class
chain and the noobj mask chain, ACT does relus/squares/accumulations,
PE broadcasts the truth-corner constants across partitions.  Scalar
partials return per core and are combined on the host (the "all-reduce"
of the loss terms).
"""
import numpy as np
import ml_dtypes
from contextlib import ExitStack

import concourse.bass as bass
import concourse.bacc as bacc
import concourse.mybir as mybir
import concourse.tile as tile
from concourse.bass_utils import run_bass_kernel_spmd

F32 = mybir.dt.float32
BF16 = mybir.dt.bfloat16
Alu = mybir.AluOpType
Ax = mybir.AxisListType
Act = mybir.ActivationFunctionType

B, NUM, CLASSES, AL = 64, 5, 20, 25
H = W = 32
HW = H * W
T = 50
NCORES = 8
BL = B // NCORES       # local batches per core
ELEM = 128             # row width (channels, padded)
CELLS = HW // NCORES   # 128 noobj cells per core

_CACHED = {}


def _build_program(blk, tn):
    """blk: number of 128-row blocks of packed (b,t) rows; tn: live truths
    of batch 63 for the noobj term."""
    nc = bacc.Bacc(enable_partition_id=False)
    tvw = 9 * blk + CLASSES
    o_pp = blk * 16                    # pf32 = xgf | p63 | tv
    o_tv = o_pp + 21
    nf = o_tv + tvw
    nb = 5 * tn + blk * CLASSES * NUM  # pbf = tbs | cls
    pf32 = nc.declare_dram_parameter("pf32", [128, nf], F32, isOutput=False)
    pbf = nc.declare_dram_parameter("pbf", [128, nb], BF16, isOutput=False)
    outd = nc.declare_dram_parameter("out", [128, 16], F32, isOutput=True)

    o_wtht, o_at = 0, 2 * blk
    o_twth, o_s2v, o_txys = 3 * blk, 5 * blk, 6 * blk
    o_vld, o_ct, o_iota = 7 * blk, 8 * blk, 9 * blk

    with tile.TileContext(nc) as tc, ExitStack() as ctx:
        pool = ctx.enter_context(tc.tile_pool(name="p", bufs=1))

        # -------- input DMAs ------------------------------------------
        itf = pool.tile([128, nf], F32)
        itb = pool.tile([128, nb], BF16)
        nc.sync.dma_start(itb[:, 0:5 * tn], pbf[:, 0:5 * tn])  # tbs
        nc.scalar.dma_start(itf[:], pf32[:])                   # xgf|p63|tv
        nc.scalar.dma_start(itb[:, 5 * tn:], pbf[:, 5 * tn:])  # cls
        xt = itf[:, 0:blk * 16]
        tbs = itb[:, 0:5 * tn]
        clsr = itb[:, 5 * tn:]
        pp = itf[:, o_pp:o_pp + 21]
        tv = itf[:, o_tv:o_tv + tvw]

        out = pool.tile([128, 16], F32)
        nc.vector.memset(out[:], 0.0)

        def tb_b(col):  # truth block [128,tn] -> [128,5,tn] (bcast anchors)
            return tbs[:, col * tn:(col + 1) * tn].rearrange(
                "p (o f) -> p o f", o=1).broadcast_to([128, NUM, tn])

        # ---------------- noobj (batch 63, this core's cells) -------------
        pv = pp[:, 0:20].rearrange("p (n c) -> p n c", c=4)
        xc, yc, wc, hc = pv[:, :, 0], pv[:, :, 1], pv[:, :, 2], pv[:, :, 3]
        corn = pool.tile([128, 25], F32)
        lo = corn[:, 0:10].rearrange("p (g n) -> p n g", g=2)    # al|au
        hi = corn[:, 10:20].rearrange("p (g n) -> p n g", g=2)   # ar|ad
        hap3 = corn[:, 20:25]
        pxy = pv[:, :, 0:2]
        pwh = pv[:, :, 2:4]
        nc.vector.scalar_tensor_tensor(lo, pwh, -0.5, pxy, Alu.mult, Alu.add)
        nc.vector.scalar_tensor_tensor(hi, pwh, 0.5, pxy, Alu.mult, Alu.add)
        # hap/1.5 = wc*hc/3  (mask test: iou>0.5 <=> inter-hat/1.5 > hap/1.5)
        nc.vector.scalar_tensor_tensor(hap3, wc, 1.0 / 3.0, hc,
                                       Alu.mult, Alu.mult)

        al, au = corn[:, 0:5], corn[:, 5:10]
        ar, ad = corn[:, 10:15], corn[:, 15:20]

        def cn_b(apv):  # corner col [128,5] -> [128,5,tn]
            return apv.broadcast_to([128, NUM, tn])

        u1 = pool.tile([128, NUM * tn], F32)
        u2 = pool.tile([128, NUM * tn], F32)
        iw = pool.tile([128, NUM * tn], F32)
        ih = pool.tile([128, NUM * tn], F32)

        def w3(tl):
            return tl[:].rearrange("p (n t) -> p n t", t=tn)

        nc.vector.tensor_tensor(w3(u1), tb_b(2), cn_b(ar), Alu.min)
        nc.vector.tensor_tensor(w3(u2), tb_b(0), cn_b(al), Alu.max)
        nc.vector.tensor_tensor(iw[:], u1[:], u2[:], Alu.subtract)
        nc.vector.tensor_tensor(w3(u1), tb_b(3), cn_b(ad), Alu.min)
        nc.vector.tensor_tensor(w3(u2), tb_b(1), cn_b(au), Alu.max)
        nc.vector.tensor_tensor(ih[:], u1[:], u2[:], Alu.subtract)
        nc.scalar.activation(ih[:], ih[:], Act.Relu)
        nc.vector.tensor_tensor(iw[:], iw[:], ih[:], Alu.mult)        # inter
        # g = inter - hat/1.5 ; mask_n = (max_t g) > hap/1.5
        nc.vector.tensor_tensor(w3(iw), w3(iw), tb_b(4), Alu.subtract)
        anyt = pool.tile([128, 6], F32)
        nc.vector.tensor_reduce(anyt[:, 0:5], w3(iw), Ax.X, Alu.max)
        nc.vector.tensor_tensor(anyt[:, 0:5], anyt[:, 0:5], hap3, Alu.is_gt)
        nc.vector.tensor_reduce(anyt[:, 5:6], anyt[:, 0:5], Ax.X, Alu.add,
                                negate=True)                      # -cnt
        oq2 = pool.tile([128, 2], F32)
        nc.scalar.square(oq2[:, 0:1], pp[:, 20:21])
        nc.scalar.activation(oq2[:, 1:2], pp[:, 20:21], Act.Square,
                             scale=float(NUM) ** 0.5)             # 5*obj^2
        nc.vector.scalar_tensor_tensor(out[:, 12:13], anyt[:, 5:6],
                                       oq2[:, 0:1], oq2[:, 1:2],
                                       Alu.mult, Alu.add)

        # ---------------- warm coords (batch 63 cells) --------------------
        scr = pool.tile([128, 20], F32)
        nc.scalar.activation(scr[:], pp[:, 0:20], Act.Square,
                             accum_out=out[:, 13:14])
        xysc = pool.tile([128, 10], F32)
        nc.scalar.activation(
            xysc[:].rearrange("p (n g) -> p n g", g=2), pv[:, :, 0:2],
            Act.Copy, accum_out=out[:, 14:15])

        # ---------------- per-(b,t) losses — DVE --------------------------
        x3 = xt.rearrange("p (c e) -> p c e", e=16)
        wv = x3[:, :, 0:5]
        hv = x3[:, :, 5:10]
        objv = x3[:, :, 10]                          # [128,blk]
        wh4 = x3[:, :, 0:10].rearrange("p c (g n) -> p c g n", g=2)
        V4 = clsr.rearrange("p (c cc n) -> p c cc n", cc=CLASSES, n=NUM)

        wtht_b = tv[:, o_wtht:o_wtht + 2 * blk].rearrange(
            "p (c g o) -> p c g o", g=2, o=1).broadcast_to([128, blk, 2, NUM])
        at_b = tv[:, o_at:o_at + blk].broadcast_to([128, blk, NUM])
        twth = tv[:, o_twth:o_twth + 2 * blk]
        s2v = tv[:, o_s2v:o_s2v + blk]
        txys = tv[:, o_txys:o_txys + blk]               # txy2*s2 (host-folded)
        vld = tv[:, o_vld:o_vld + blk]
        ct_b = tv[:, o_ct:o_ct + blk].broadcast_to([128, blk, CLASSES])
        iota_b = tv[:, o_iota:o_iota + CLASSES].rearrange(
            "p (o e) -> p o e", o=1).broadcast_to([128, blk, CLASSES])

        t1 = pool.tile([128, blk * NUM], F32)
        t2 = pool.tile([128, blk * 2 * NUM], F32)
        t3 = pool.tile([128, blk * NUM], F32)
        t4 = pool.tile([128, blk * NUM], F32)
        mx = pool.tile([128, blk], F32)
        wbhb = pool.tile([128, blk * 2], F32)
        cdf = pool.tile([128, blk], F32)
        t6 = pool.tile([128, blk * 2 * NUM], F32)
        t6b = pool.tile([128, blk * CLASSES * NUM], BF16)
        t1b = pool.tile([128, blk * NUM], BF16)
        oh = pool.tile([128, blk * CLASSES], F32)
        cbb = pool.tile([128, blk * CLASSES], BF16)
        cb = pool.tile([128, blk * CLASSES], F32)
        csq = pool.tile([128, blk * CLASSES], F32)
        cda = pool.tile([128, blk], F32)

        def b5(tl):
            return tl[:].rearrange("p (c n) -> p c n", n=NUM)

        def b20(tl):
            return tl[:].rearrange("p (c e) -> p c e", e=CLASSES)

        t12 = t2[:].rearrange("p (c g n) -> p c g n", g=2, n=NUM)
        nc.vector.tensor_tensor(t12, wh4, wtht_b, Alu.min)
        nc.vector.tensor_tensor(b5(t3), t12[:, :, 0], t12[:, :, 1],
                                Alu.mult)                          # inter
        # argmax_n inter/union == argmax_n inter/(A_n+B): x/(S-x) is
        # monotone in x/S, so the -inter term of the union drops out.
        nc.vector.tensor_tensor(b5(t4), wv, hv, Alu.mult)
        nc.vector.tensor_tensor(b5(t4), b5(t4), at_b, Alu.add)     # A+B
        nc.vector.reciprocal(t4[:], t4[:])
        nc.vector.tensor_tensor(t3[:], t3[:], t4[:], Alu.mult)     # score
        nc.vector.tensor_reduce(mx[:], b5(t3), Ax.X, Alu.max)
        nc.vector.tensor_tensor(b5(t1b), b5(t3),
                                mx[:].broadcast_to([128, blk, NUM]), Alu.is_ge)
        m4 = t1b[:].rearrange("p (c o n) -> p c o n", o=1, n=NUM)
        t6wh = t6[:, 0:blk * 2 * NUM].rearrange(
            "p (c g n) -> p c g n", g=2, n=NUM)
        nc.vector.tensor_tensor(
            t6wh, wh4, m4.broadcast_to([128, blk, 2, NUM]), Alu.mult)
        nc.vector.tensor_reduce(
            wbhb[:].rearrange("p (c g) -> p c g", g=2), t6wh, Ax.X, Alu.add)
        # coords: ((tw-wb)^2 + (th-hb)^2)*s2 + txy2*s2
        nc.vector.tensor_tensor(wbhb[:], twth, wbhb[:], Alu.subtract)
        nc.vector.tensor_tensor(wbhb[:], wbhb[:], wbhb[:], Alu.mult)
        nc.vector.tensor_reduce(cdf[:], wbhb[:].rearrange(
            "p (c g) -> p c g", g=2), Ax.X, Alu.add)
        nc.vector.tensor_tensor(cdf[:], cdf[:], s2v, Alu.mult)
        nc.vector.tensor_tensor(out[:, 0:blk], cdf[:], txys, Alu.add)
        # obj: (1-obj)^2; pad rows contribute exactly 1 (host-subtracted)
        nc.scalar.activation(out[:, 4:4 + blk], objv, Act.Square,
                             bias=1.0, scale=-1.0)
        # classes (select in bf16 2x mode, squares accumulated on ACT)
        nc.vector.tensor_tensor(b20(oh), iota_b, ct_b, Alu.is_equal)
        t64 = t6b[:].rearrange("p (c cc n) -> p c cc n", cc=CLASSES, n=NUM)
        nc.vector.tensor_tensor(
            t64, V4, m4.broadcast_to([128, blk, CLASSES, NUM]), Alu.mult)
        with nc.allow_low_precision("bf16 single-anchor class select"):
            nc.vector.tensor_reduce(b20(cbb), t64, Ax.X, Alu.add)
        nc.vector.scalar_tensor_tensor(cb[:], cbb[:], -1.0, oh[:],
                                       Alu.mult, Alu.add)
        # pad rows have zero class data and ct=0 -> each contributes exactly
        # 1.0 here; the host subtracts the pad count.
        nc.scalar.activation(csq[:], cb[:], Act.Square,
                             accum_out=out[:, 8:9])

        nc.sync.dma_start(outd[:], out[:])
    nc.finalize()
    return nc


def _prep(x, truth, anchors):
    f32 = np.float32
    x = np.ascontiguousarray(x, f32)
    truth = np.ascontiguousarray(truth, f32)
    anchors = np.asarray(anchors, f32)

    wt, ht = truth[..., 2], truth[..., 3]
    valid = np.cumprod((wt >= 1e-5).astype(f32), axis=1, dtype=f32)
    i = np.clip((truth[..., 0] * f32(W)).astype(np.int32), 0, W - 1)
    j = np.clip((truth[..., 1] * f32(H)).astype(np.int32), 0, H - 1)
    lin = j * W + i                                        # [B,T]
    tx = i.astype(f32) / f32(W)
    ty = j.astype(f32) / f32(H)
    tw = np.exp(wt) * anchors[2 * (NUM - 1)] / f32(W)
    th = np.exp(ht) * anchors[2 * (NUM - 1) + 1] / f32(H)
    at = wt * ht
    scale = (f32(2.0) - at).astype(f32)
    s2 = scale * scale
    txy2 = tx * tx + ty * ty
    ct = np.clip(truth[..., 4].astype(np.int32), 0, CLASSES - 1).astype(f32)

    # gathered rows at the truth cells: [B, T, 16] f32 + [B, T, 100] bf16
    x5 = x.reshape(B, NUM, AL, HW)
    g = x5[np.arange(B)[:, None], :, :, lin]               # [B,T,NUM,AL]
    rows = np.zeros((B, T, 16), f32)
    rows[:, :, 0:5] = g[:, :, :, 2]
    rows[:, :, 5:10] = g[:, :, :, 3]
    rows[:, :, 10] = g[:, :, 0, 4]
    crows = np.ascontiguousarray(
        g[:, :, :, 5:25].transpose(0, 1, 3, 2).reshape(B, T, NUM * CLASSES)
    ).astype(ml_dtypes.bfloat16)

    # fields: wt ht at tw th s2 txy2*s2 _ ct  (order matches packing below)
    fields = np.stack([wt, ht, at, tw, th, s2, txy2 * s2, valid, ct],
                      axis=-1)                             # [B,T,9]

    vmask = valid.astype(bool)
    nv_core = [int(vmask[BL * c:BL * (c + 1)].sum()) for c in range(NCORES)]
    blk = max(1, -(-max(nv_core) // 128))

    # batch-63 truths with zero area can't set the noobj mask -> prune
    t63 = truth[B - 1]
    live = (t63[:, 2] * t63[:, 3]) > 0.0
    tn = max(1, int(live.sum()))
    t63l = t63[live][:tn]
    bl_ = t63l[:, 0] - f32(0.5) * t63l[:, 2]
    bu_ = t63l[:, 1] - f32(0.5) * t63l[:, 3]
    br_ = t63l[:, 0] + f32(0.5) * t63l[:, 2]
    bd_ = t63l[:, 1] + f32(0.5) * t63l[:, 3]
    hat3 = (t63l[:, 2] * t63l[:, 3]) / f32(3.0)
    tbrow = np.concatenate([bl_, bu_, br_, bd_, hat3]).astype(ml_dtypes.bfloat16)

    xp63 = x[B - 1].reshape(NUM * AL, HW)

    def fold(vec, n):
        v = np.zeros(blk * 128, f32)
        v[:n] = vec
        return v.reshape(blk, 128).T                       # [128, blk]

    in_maps = []
    pads = []
    b63_pos = None
    for c in range(NCORES):
        bs = slice(BL * c, BL * (c + 1))
        cells = slice(CELLS * c, CELLS * (c + 1))
        m = vmask[bs]                                      # [BL, T]
        n = int(m.sum())

        rw = rows[bs][m]                                   # [n, 16]
        pad = np.broadcast_to(rw[0:1] if n else np.zeros((1, 16), f32),
                              (blk * 128 - n, 16))
        rw512 = np.concatenate([rw, np.ascontiguousarray(pad)])
        rw512[n:, 10] = 0.0                    # pad obj -> (1-0)^2 = 1 exact
        xgc = np.ascontiguousarray(
            rw512.reshape(blk, 128, 16).transpose(1, 0, 2).reshape(128, -1))
        cw = crows[bs][m]                                  # [n, 100] bf16
        cpad = np.zeros((blk * 128 - n, 100), ml_dtypes.bfloat16)
        cw512 = np.concatenate([cw, cpad])
        clsc = np.ascontiguousarray(
            cw512.reshape(blk, 128, 100).transpose(1, 0, 2).reshape(128, -1))

        fc = fields[bs][m]                                 # [n, 9]
        tvw = 9 * blk + CLASSES
        tv = np.zeros((128, tvw), f32)
        tv[:, 0:2 * blk:2] = fold(fc[:, 0], n)             # wt (interleaved)
        tv[:, 1:2 * blk:2] = fold(fc[:, 1], n)             # ht
        tv[:, 2 * blk:3 * blk] = fold(fc[:, 2], n)         # at
        tv[:, 3 * blk:5 * blk:2] = fold(fc[:, 3], n)       # tw
        tv[:, 3 * blk + 1:5 * blk:2] = fold(fc[:, 4], n)   # th
        tv[:, 5 * blk:6 * blk] = fold(fc[:, 5], n)         # s2 (packed->vld=1)
        tv[:, 6 * blk:7 * blk] = fold(fc[:, 6], n)         # txy2*s2
        tv[:, 7 * blk:8 * blk] = fold(np.ones(n, f32), n)  # vld
        tv[:, 8 * blk:9 * blk] = fold(fc[:, 8], n)         # ct
        tv[:, 9 * blk:9 * blk + CLASSES] = np.arange(CLASSES, dtype=f32)

        p63 = np.empty((128, 21), f32)
        for an in range(NUM):
            for cc in range(4):
                p63[:, an * 4 + cc] = xp63[AL * an + cc, cells]
        p63[:, 20] = xp63[4, cells]

        pads.append(blk * 128 - n)
        if c == NCORES - 1:
            start = int(m[:BL - 1].sum())
            b63_pos = (start, int(m[BL - 1].sum()))

        in_maps.append({
            "pf32": np.ascontiguousarray(
                np.concatenate([xgc, p63, tv], axis=1)),
            "pbf": np.ascontiguousarray(np.concatenate(
                [np.broadcast_to(tbrow, (128, 5 * tn)), clsc], axis=1)),
        })
    return in_maps, blk, tn, (b63_pos, pads)


def _combine(results, blk, b63_info):
    b63_pos, pads = b63_info
    npad = float(sum(pads))
    obj = sum(float(r["out"][:, 4:4 + blk].sum(dtype=np.float64))
              for r in results) - npad
    cls = sum(float(r["out"][:, 8].sum(dtype=np.float64))
              for r in results) - npad
    noobj = sum(float(r["out"][:, 12].sum(dtype=np.float64)) for r in results)
    sq = sum(float(r["out"][:, 13].sum(dtype=np.float64)) for r in results)
    xy = sum(float(r["out"][:, 14].sum(dtype=np.float64)) for r in results)
    r7 = results[NCORES - 1]["out"]
    start, cnt = b63_pos
    coord63 = 0.0
    for idx in range(start, start + cnt):
        coord63 += float(r7[idx % 128, idx // 128])
    warm = 0.01 * (sq - xy + 0.5 * NUM * HW)
    return np.float32(obj + noobj + warm + coord63 + cls)


def kernel(x, truth, anchors, **_):
    in_maps, blk, tn, b63_pos = _prep(x, truth, anchors)
    key = (blk, tn)
    if key not in _CACHED:
        _CACHED[key] = _build_program(blk, tn)
    nc = _CACHED[key]
    res = run_bass_kernel_spmd(nc, in_maps, list(range(NCORES)))
    return _combine(res.results, blk, b63_pos)


# revision 3
# speedup vs baseline: 1.0187x; 1.0135x over previous
"""YOLOv2-style loss (nn_CostYoloV2) on 8 Trainium2 NeuronCores — v2.

Data-parallel over batch (8 batches/core); the noobj term (batch 63 only)
is sharded over the 1024 spatial cells (128 cells/core).

Layout strategy: the truth-cell gather indices are pure host data
(computed from `truth` alone), so the host packs the valid (batch, truth)
rows per core into a [128 part, BLK block, 128 ch] tile and the device
consumes them directly — measured on-device alternatives (streaming all
23 used channels of x + gpsimd indirect_copy + PE transpose: ~26us of DMA
at the 2-queue HWDGE ceiling; swdge dma_gather: ~15us of descriptor
generation; trn2's Pool engine has no elementwise ALU) are all slower.
Rows with valid=0 and all-zero truth slots are pruned host-side (their
loss contributions are provably zero), shrinking both DMA and DVE work.

Per-(b,t) row layout (128 f32): [w x5 anchors | h x5 | obj0 | pad x5 |
cls x20 anchor-innermost | pad x12].  Per-(b,t) scalars ride in a side
tile.  Engine split: DVE runs both the per-(b,t) iou/argmax/coord/class
chain and the noobj mask chain, ACT does relus/squares/accumulations,
PE broadcasts the truth-corner constants across partitions.  Scalar
partials return per core and are combined on the host (the "all-reduce"
of the loss terms).
"""
import numpy as np
import ml_dtypes
from contextlib import ExitStack

import concourse.bass as bass
import concourse.bacc as bacc
import concourse.mybir as mybir
import concourse.tile as tile
from concourse.bass_utils import run_bass_kernel_spmd

F32 = mybir.dt.float32
BF16 = mybir.dt.bfloat16
Alu = mybir.AluOpType
Ax = mybir.AxisListType
Act = mybir.ActivationFunctionType

B, NUM, CLASSES, AL = 64, 5, 20, 25
H = W = 32
HW = H * W
T = 50
NCORES = 8
BL = B // NCORES       # local batches per core
ELEM = 128             # row width (channels, padded)
CELLS = HW // NCORES   # 128 noobj cells per core

_CACHED = {}


def _build_program(blk, tn):
    """blk: number of 128-row blocks of packed (b,t) rows; tn: live truths
    of batch 63 for the noobj term."""
    nc = bacc.Bacc(enable_partition_id=False)
    tvw = 9 * blk + CLASSES
    o_pp = 0                           # pf32 = p63 | tv | xgf
    o_tv = 21
    o_xg = o_tv + tvw
    nf = o_xg + blk * 16
    nb = 5 * tn + blk * CLASSES * NUM  # pbf = tbs | cls
    pf32 = nc.declare_dram_parameter("pf32", [128, nf], F32, isOutput=False)
    pbf = nc.declare_dram_parameter("pbf", [128, nb], BF16, isOutput=False)
    outd = nc.declare_dram_parameter("out", [128, 16], F32, isOutput=True)

    o_wtht, o_at = 0, 2 * blk
    o_twth, o_s2v, o_txys = 3 * blk, 5 * blk, 6 * blk
    o_vld, o_ct, o_iota = 7 * blk, 8 * blk, 9 * blk

    with tile.TileContext(nc) as tc, ExitStack() as ctx:
        pool = ctx.enter_context(tc.tile_pool(name="p", bufs=1))

        # -------- input DMAs ------------------------------------------
        itf = pool.tile([128, nf], F32)
        itb = pool.tile([128, nb], BF16)
        nc.sync.dma_start(itb[:, 0:5 * tn], pbf[:, 0:5 * tn])  # tbs
        nc.scalar.dma_start(itf[:, 0:o_xg], pf32[:, 0:o_xg])   # p63|tv
        nc.scalar.dma_start(itf[:, o_xg:], pf32[:, o_xg:])     # xgf
        nc.scalar.dma_start(itb[:, 5 * tn:], pbf[:, 5 * tn:])  # cls
        xt = itf[:, o_xg:o_xg + blk * 16]
        tbs = itb[:, 0:5 * tn]
        clsr = itb[:, 5 * tn:]
        pp = itf[:, o_pp:o_pp + 21]
        tv = itf[:, o_tv:o_tv + tvw]

        out = pool.tile([128, 16], F32)
        nc.vector.memset(out[:], 0.0)

        def tb_b(col):  # truth block [128,tn] -> [128,5,tn] (bcast anchors)
            return tbs[:, col * tn:(col + 1) * tn].rearrange(
                "p (o f) -> p o f", o=1).broadcast_to([128, NUM, tn])

        # ---------------- noobj (batch 63, this core's cells) -------------
        pv = pp[:, 0:20].rearrange("p (n c) -> p n c", c=4)
        xc, yc, wc, hc = pv[:, :, 0], pv[:, :, 1], pv[:, :, 2], pv[:, :, 3]
        corn = pool.tile([128, 25], F32)
        lo = corn[:, 0:10].rearrange("p (g n) -> p n g", g=2)    # al|au
        hi = corn[:, 10:20].rearrange("p (g n) -> p n g", g=2)   # ar|ad
        hap3 = corn[:, 20:25]
        pxy = pv[:, :, 0:2]
        pwh = pv[:, :, 2:4]
        nc.vector.scalar_tensor_tensor(lo, pwh, -0.5, pxy, Alu.mult, Alu.add)
        nc.vector.scalar_tensor_tensor(hi, pwh, 0.5, pxy, Alu.mult, Alu.add)
        # hap/1.5 = wc*hc/3  (mask test: iou>0.5 <=> inter-hat/1.5 > hap/1.5)
        nc.vector.scalar_tensor_tensor(hap3, wc, 1.0 / 3.0, hc,
                                       Alu.mult, Alu.mult)

        al, au = corn[:, 0:5], corn[:, 5:10]
        ar, ad = corn[:, 10:15], corn[:, 15:20]

        def cn_b(apv):  # corner col [128,5] -> [128,5,tn]
            return apv.broadcast_to([128, NUM, tn])

        u1 = pool.tile([128, NUM * tn], F32)
        u2 = pool.tile([128, NUM * tn], F32)
        iw = pool.tile([128, NUM * tn], F32)
        ih = pool.tile([128, NUM * tn], F32)

        def w3(tl):
            return tl[:].rearrange("p (n t) -> p n t", t=tn)

        nc.vector.tensor_tensor(w3(u1), tb_b(2), cn_b(ar), Alu.min)
        nc.vector.tensor_tensor(w3(u2), tb_b(0), cn_b(al), Alu.max)
        nc.vector.tensor_tensor(iw[:], u1[:], u2[:], Alu.subtract)
        nc.vector.tensor_tensor(w3(u1), tb_b(3), cn_b(ad), Alu.min)
        nc.vector.tensor_tensor(w3(u2), tb_b(1), cn_b(au), Alu.max)
        nc.vector.tensor_tensor(ih[:], u1[:], u2[:], Alu.subtract)
        nc.scalar.activation(ih[:], ih[:], Act.Relu)
        nc.vector.tensor_tensor(iw[:], iw[:], ih[:], Alu.mult)        # inter
        # g = inter - hat/1.5 ; mask_n = (max_t g) > hap/1.5
        nc.vector.tensor_tensor(w3(iw), w3(iw), tb_b(4), Alu.subtract)
        anyt = pool.tile([128, 6], F32)
        nc.vector.tensor_reduce(anyt[:, 0:5], w3(iw), Ax.X, Alu.max)
        nc.vector.tensor_tensor(anyt[:, 0:5], anyt[:, 0:5], hap3, Alu.is_gt)
        nc.vector.tensor_reduce(anyt[:, 5:6], anyt[:, 0:5], Ax.X, Alu.add,
                                negate=True)                      # -cnt
        oq2 = pool.tile([128, 2], F32)
        nc.scalar.square(oq2[:, 0:1], pp[:, 20:21])
        nc.scalar.activation(oq2[:, 1:2], pp[:, 20:21], Act.Square,
                             scale=float(NUM) ** 0.5)             # 5*obj^2
        nc.vector.scalar_tensor_tensor(out[:, 12:13], anyt[:, 5:6],
                                       oq2[:, 0:1], oq2[:, 1:2],
                                       Alu.mult, Alu.add)

        # ---------------- warm coords (batch 63 cells) --------------------
        scr = pool.tile([128, 20], F32)
        nc.scalar.activation(scr[:], pp[:, 0:20], Act.Square,
                             accum_out=out[:, 13:14])
        xysc = pool.tile([128, 10], F32)
        nc.scalar.activation(
            xysc[:].rearrange("p (n g) -> p n g", g=2), pv[:, :, 0:2],
            Act.Copy, accum_out=out[:, 14:15])

        # ---------------- per-(b,t) losses — DVE --------------------------
        x3 = xt.rearrange("p (c e) -> p c e", e=16)
        wv = x3[:, :, 0:5]
        hv = x3[:, :, 5:10]
        objv = x3[:, :, 10]                          # [128,blk]
        wh4 = x3[:, :, 0:10].rearrange("p c (g n) -> p c g n", g=2)
        V4 = clsr.rearrange("p (c cc n) -> p c cc n", cc=CLASSES, n=NUM)

        wtht_b = tv[:, o_wtht:o_wtht + 2 * blk].rearrange(
            "p (c g o) -> p c g o", g=2, o=1).broadcast_to([128, blk, 2, NUM])
        at_b = tv[:, o_at:o_at + blk].broadcast_to([128, blk, NUM])
        twth = tv[:, o_twth:o_twth + 2 * blk]
        s2v = tv[:, o_s2v:o_s2v + blk]
        txys = tv[:, o_txys:o_txys + blk]               # txy2*s2 (host-folded)
        vld = tv[:, o_vld:o_vld + blk]
        ct_b = tv[:, o_ct:o_ct + blk].broadcast_to([128, blk, CLASSES])
        iota_b = tv[:, o_iota:o_iota + CLASSES].rearrange(
            "p (o e) -> p o e", o=1).broadcast_to([128, blk, CLASSES])

        t1 = pool.tile([128, blk * NUM], F32)
        t2 = pool.tile([128, blk * 2 * NUM], F32)
        t3 = pool.tile([128, blk * NUM], F32)
        t4 = pool.tile([128, blk * NUM], F32)
        mx = pool.tile([128, blk], F32)
        wbhb = pool.tile([128, blk * 2], F32)
        cdf = pool.tile([128, blk], F32)
        t6 = pool.tile([128, blk * 2 * NUM], F32)
        t6b = pool.tile([128, blk * CLASSES * NUM], BF16)
        t1b = pool.tile([128, blk * NUM], BF16)
        oh = pool.tile([128, blk * CLASSES], F32)
        cbb = pool.tile([128, blk * CLASSES], BF16)
        cb = pool.tile([128, blk * CLASSES], F32)
        csq = pool.tile([128, blk * CLASSES], F32)
        cda = pool.tile([128, blk], F32)

        def b5(tl):
            return tl[:].rearrange("p (c n) -> p c n", n=NUM)

        def b20(tl):
            return tl[:].rearrange("p (c e) -> p c e", e=CLASSES)

        t12 = t2[:].rearrange("p (c g n) -> p c g n", g=2, n=NUM)
        nc.vector.tensor_tensor(t12, wh4, wtht_b, Alu.min)
        nc.vector.tensor_tensor(b5(t3), t12[:, :, 0], t12[:, :, 1],
                                Alu.mult)                          # inter
        # argmax_n inter/union == argmax_n inter/(A_n+B): x/(S-x) is
        # monotone in x/S, so the -inter term of the union drops out.
        nc.vector.tensor_tensor(b5(t4), wv, hv, Alu.mult)
        nc.vector.tensor_tensor(b5(t4), b5(t4), at_b, Alu.add)     # A+B
        nc.vector.reciprocal(t4[:], t4[:])
        nc.vector.tensor_tensor(t3[:], t3[:], t4[:], Alu.mult)     # score
        nc.vector.tensor_reduce(mx[:], b5(t3), Ax.X, Alu.max)
        nc.vector.tensor_tensor(b5(t1b), b5(t3),
                                mx[:].broadcast_to([128, blk, NUM]), Alu.is_ge)
        m4 = t1b[:].rearrange("p (c o n) -> p c o n", o=1, n=NUM)
        t6wh = t6[:, 0:blk * 2 * NUM].rearrange(
            "p (c g n) -> p c g n", g=2, n=NUM)
        nc.vector.tensor_tensor(
            t6wh, wh4, m4.broadcast_to([128, blk, 2, NUM]), Alu.mult)
        nc.vector.tensor_reduce(
            wbhb[:].rearrange("p (c g) -> p c g", g=2), t6wh, Ax.X, Alu.add)
        # coords: ((tw-wb)^2 + (th-hb)^2)*s2 + txy2*s2
        nc.vector.tensor_tensor(wbhb[:], twth, wbhb[:], Alu.subtract)
        nc.vector.tensor_tensor(wbhb[:], wbhb[:], wbhb[:], Alu.mult)
        nc.vector.tensor_reduce(cdf[:], wbhb[:].rearrange(
            "p (c g) -> p c g", g=2), Ax.X, Alu.add)
        nc.vector.tensor_tensor(cdf[:], cdf[:], s2v, Alu.mult)
        nc.vector.tensor_tensor(out[:, 0:blk], cdf[:], txys, Alu.add)
        # obj: (1-obj)^2; pad rows contribute exactly 1 (host-subtracted)
        nc.scalar.activation(out[:, 4:4 + blk], objv, Act.Square,
                             bias=1.0, scale=-1.0)
        # classes (select in bf16 2x mode, squares accumulated on ACT)
        nc.vector.tensor_tensor(b20(oh), iota_b, ct_b, Alu.is_equal)
        t64 = t6b[:].rearrange("p (c cc n) -> p c cc n", cc=CLASSES, n=NUM)
        nc.vector.tensor_tensor(
            t64, V4, m4.broadcast_to([128, blk, CLASSES, NUM]), Alu.mult)
        with nc.allow_low_precision("bf16 single-anchor class select"):
            nc.vector.tensor_reduce(b20(cbb), t64, Ax.X, Alu.add)
        nc.vector.scalar_tensor_tensor(cb[:], cbb[:], -1.0, oh[:],
                                       Alu.mult, Alu.add)
        # pad rows have zero class data and ct=0 -> each contributes exactly
        # 1.0 here; the host subtracts the pad count.
        nc.scalar.activation(csq[:], cb[:], Act.Square,
                             accum_out=out[:, 8:9])

        nc.sync.dma_start(outd[:], out[:])
    nc.finalize()
    return nc


def _prep(x, truth, anchors):
    f32 = np.float32
    x = np.ascontiguousarray(x, f32)
    truth = np.ascontiguousarray(truth, f32)
    anchors = np.asarray(anchors, f32)

    wt, ht = truth[..., 2], truth[..., 3]
    valid = np.cumprod((wt >= 1e-5).astype(f32), axis=1, dtype=f32)
    i = np.clip((truth[..., 0] * f32(W)).astype(np.int32), 0, W - 1)
    j = np.clip((truth[..., 1] * f32(H)).astype(np.int32), 0, H - 1)
    lin = j * W + i                                        # [B,T]
    tx = i.astype(f32) / f32(W)
    ty = j.astype(f32) / f32(H)
    tw = np.exp(wt) * anchors[2 * (NUM - 1)] / f32(W)
    th = np.exp(ht) * anchors[2 * (NUM - 1) + 1] / f32(H)
    at = wt * ht
    scale = (f32(2.0) - at).astype(f32)
    s2 = scale * scale
    txy2 = tx * tx + ty * ty
    ct = np.clip(truth[..., 4].astype(np.int32), 0, CLASSES - 1).astype(f32)

    # gathered rows at the truth cells: [B, T, 16] f32 + [B, T, 100] bf16
    x5 = x.reshape(B, NUM, AL, HW)
    g = x5[np.arange(B)[:, None], :, :, lin]               # [B,T,NUM,AL]
    rows = np.zeros((B, T, 16), f32)
    rows[:, :, 0:5] = g[:, :, :, 2]
    rows[:, :, 5:10] = g[:, :, :, 3]
    rows[:, :, 10] = g[:, :, 0, 4]
    crows = np.ascontiguousarray(
        g[:, :, :, 5:25].transpose(0, 1, 3, 2).reshape(B, T, NUM * CLASSES)
    ).astype(ml_dtypes.bfloat16)

    # fields: wt ht at tw th s2 txy2*s2 _ ct  (order matches packing below)
    fields = np.stack([wt, ht, at, tw, th, s2, txy2 * s2, valid, ct],
                      axis=-1)                             # [B,T,9]

    vmask = valid.astype(bool)
    nv_core = [int(vmask[BL * c:BL * (c + 1)].sum()) for c in range(NCORES)]
    blk = max(1, -(-max(nv_core) // 128))

    # batch-63 truths with zero area can't set the noobj mask -> prune
    t63 = truth[B - 1]
    live = (t63[:, 2] * t63[:, 3]) > 0.0
    tn = max(1, int(live.sum()))
    t63l = t63[live][:tn]
    bl_ = t63l[:, 0] - f32(0.5) * t63l[:, 2]
    bu_ = t63l[:, 1] - f32(0.5) * t63l[:, 3]
    br_ = t63l[:, 0] + f32(0.5) * t63l[:, 2]
    bd_ = t63l[:, 1] + f32(0.5) * t63l[:, 3]
    hat3 = (t63l[:, 2] * t63l[:, 3]) / f32(3.0)
    tbrow = np.concatenate([bl_, bu_, br_, bd_, hat3]).astype(ml_dtypes.bfloat16)

    xp63 = x[B - 1].reshape(NUM * AL, HW)

    def fold(vec, n):
        v = np.zeros(blk * 128, f32)
        v[:n] = vec
        return v.reshape(blk, 128).T                       # [128, blk]

    in_maps = []
    pads = []
    b63_pos = None
    for c in range(NCORES):
        bs = slice(BL * c, BL * (c + 1))
        cells = slice(CELLS * c, CELLS * (c + 1))
        m = vmask[bs]                                      # [BL, T]
        n = int(m.sum())

        rw = rows[bs][m]                                   # [n, 16]
        pad = np.broadcast_to(rw[0:1] if n else np.zeros((1, 16), f32),
                              (blk * 128 - n, 16))
        rw512 = np.concatenate([rw, np.ascontiguousarray(pad)])
        rw512[n:, 10] = 0.0                    # pad obj -> (1-0)^2 = 1 exact
        xgc = np.ascontiguousarray(
            rw512.reshape(blk, 128, 16).transpose(1, 0, 2).reshape(128, -1))
        cw = crows[bs][m]                                  # [n, 100] bf16
        cpad = np.zeros((blk * 128 - n, 100), ml_dtypes.bfloat16)
        cw512 = np.concatenate([cw, cpad])
        clsc = np.ascontiguousarray(
            cw512.reshape(blk, 128, 100).transpose(1, 0, 2).reshape(128, -1))

        fc = fields[bs][m]                                 # [n, 9]
        tvw = 9 * blk + CLASSES
        tv = np.zeros((128, tvw), f32)
        tv[:, 0:2 * blk:2] = fold(fc[:, 0], n)             # wt (interleaved)
        tv[:, 1:2 * blk:2] = fold(fc[:, 1], n)             # ht
        tv[:, 2 * blk:3 * blk] = fold(fc[:, 2], n)         # at
        tv[:, 3 * blk:5 * blk:2] = fold(fc[:, 3], n)       # tw
        tv[:, 3 * blk + 1:5 * blk:2] = fold(fc[:, 4], n)   # th
        tv[:, 5 * blk:6 * blk] = fold(fc[:, 5], n)         # s2 (packed->vld=1)
        tv[:, 6 * blk:7 * blk] = fold(fc[:, 6], n)         # txy2*s2
        tv[:, 7 * blk:8 * blk] = fold(np.ones(n, f32), n)  # vld
        tv[:, 8 * blk:9 * blk] = fold(fc[:, 8], n)         # ct
        tv[:, 9 * blk:9 * blk + CLASSES] = np.arange(CLASSES, dtype=f32)

        p63 = np.empty((128, 21), f32)
        for an in range(NUM):
            for cc in range(4):
                p63[:, an * 4 + cc] = xp63[AL * an + cc, cells]
        p63[:, 20] = xp63[4, cells]

        pads.append(blk * 128 - n)
        if c == NCORES - 1:
            start = int(m[:BL - 1].sum())
            b63_pos = (start, int(m[BL - 1].sum()))

        in_maps.append({
            "pf32": np.ascontiguousarray(
                np.concatenate([p63, tv, xgc], axis=1)),
            "pbf": np.ascontiguousarray(np.concatenate(
                [np.broadcast_to(tbrow, (128, 5 * tn)), clsc], axis=1)),
        })
    return in_maps, blk, tn, (b63_pos, pads)


def _combine(results, blk, b63_info):
    b63_pos, pads = b63_info
    npad = float(sum(pads))
    obj = sum(float(r["out"][:, 4:4 + blk].sum(dtype=np.float64))
              for r in results) - npad
    cls = sum(float(r["out"][:, 8].sum(dtype=np.float64))
              for r in results) - npad
    noobj = sum(float(r["out"][:, 12].sum(dtype=np.float64)) for r in results)
    sq = sum(float(r["out"][:, 13].sum(dtype=np.float64)) for r in results)
    xy = sum(float(r["out"][:, 14].sum(dtype=np.float64)) for r in results)
    r7 = results[NCORES - 1]["out"]
    start, cnt = b63_pos
    coord63 = 0.0
    for idx in range(start, start + cnt):
        coord63 += float(r7[idx % 128, idx // 128])
    warm = 0.01 * (sq - xy + 0.5 * NUM * HW)
    return np.float32(obj + noobj + warm + coord63 + cls)


def kernel(x, truth, anchors, **_):
    in_maps, blk, tn, b63_pos = _prep(x, truth, anchors)
    key = (blk, tn)
    if key not in _CACHED:
        _CACHED[key] = _build_program(blk, tn)
    nc = _CACHED[key]
    res = run_bass_kernel_spmd(nc, in_maps, list(range(NCORES)))
    return _combine(res.results, blk, b63_pos)
